# revision 14
# baseline (speedup 1.0000x reference)
"""PINN Navier-Stokes residual kernel for trn2 (8 cores, data parallel).

Strategy: propagate a 13-stream first/second/third-order Taylor jet
{v,x,y,t,xx,xy,yy,xt,yt,xxx,xxy,xyy,yyy} through the 3-128x8-2 tanh MLP.
Activations live as [128 hidden, Npts free] tiles; each hidden layer is
28 fp16 matmuls per 512-pt chunk (addends of each stream kept separate and
accumulated in PSUM), followed by fp16 elementwise chain-rule updates split
across Vector/GpSimd/Scalar engines.  Layer 0 and the final jet->(u,v,f_u,f_v)
stage are fp32.
"""

import os
import numpy as np
from contextlib import ExitStack

import concourse.bass as bass
import concourse.bacc as bacc
import concourse.tile as tile
from concourse import mybir
from concourse.bass_utils import run_bass_kernel_spmd

F32 = mybir.dt.float32
F16 = mybir.dt.float16
OP = mybir.AluOpType
AF = mybir.ActivationFunctionType

N_CORES = 8
N = 32768
NLOC = N // N_CORES      # 4096 points per core
BLK = 1024               # elementwise batch
NBLK = NLOC // BLK       # 4
CH = 512                 # matmul / psum chunk
NCH = BLK // CH          # 2
H = 128
PB = NLOC // H           # 32 free cols in final [128, PB] tiles

STREAMS = ["v", "x", "y", "t", "xx", "xy", "yy", "xt", "yt",
           "xxx", "xxy", "xyy", "yyy"]
# addend counts for A produced by hidden layers 1..7
# (second-deriv streams merged to one addend via elementwise adds)
NADD = {"v": 1, "x": 1, "y": 1, "t": 1, "xx": 1, "xy": 1, "yy": 1,
        "xt": 1, "yt": 1, "xxx": 3, "xxy": 4, "xyy": 4, "yyy": 3}
ZCOPY = ("x", "y", "t", "xx", "xy", "yy")
# streams whose z is consumed directly from PSUM (-> addend index)
PSUM_DIRECT = {"xt": 0, "yt": 0, "xxx": 2, "xxy": 3, "xyy": 3, "yyy": 2}


def _build():
    nc = bacc.Bacc(None, target_bir_lowering=False)

    pts_d = nc.declare_dram_parameter("pts", [3, NLOC], F32, False)
    w0_d = nc.declare_dram_parameter("W0f", [3, H], F32, False)
    wh_d = {li: nc.declare_dram_parameter(f"Wh{li}", [H, H], F16, False)
            for li in range(1, 8)}
    b_d = {li: nc.declare_dram_parameter(f"bb{li}", [H, 1], F32, False)
           for li in range(0, 8)}
    c0_d = nc.declare_dram_parameter("c0", [H, 12], F32, False)
    w8c_d = nc.declare_dram_parameter("W8C", [H, 16 * 13], F16, False)
    b8_d = nc.declare_dram_parameter("b8v", [H, 1], F32, False)
    lam_d = nc.declare_dram_parameter("lam", [H, 4], F32, False)
    out_d = {k: nc.declare_dram_parameter(k, [H, PB], F32, True)
             for k in ["uo", "vo", "fuo", "fvo"]}

    with tile.TileContext(nc) as tc, ExitStack() as ctx:
        cpool = ctx.enter_context(tc.tile_pool(name="consts", bufs=1))
        apool = ctx.enter_context(tc.tile_pool(name="A", bufs=2))
        chain = ctx.enter_context(tc.tile_pool(name="chain", bufs=1))
        zcp = ctx.enter_context(tc.tile_pool(name="zc", bufs=1))
        misc = ctx.enter_context(tc.tile_pool(name="misc", bufs=1))
        fpool = ctx.enter_context(tc.tile_pool(name="fin", bufs=1))
        zpool = ctx.enter_context(
            tc.tile_pool(name="psum_z", bufs=7, space="PSUM"))
        z8pool = ctx.enter_context(
            tc.tile_pool(name="psum_z8", bufs=1, space="PSUM"))

        def ctile(name, shape, dt):
            t = cpool.tile(shape, dt, name=name, tag=name)
            return t

        w0s = ctile("w0s", [3, H], F32)
        nc.sync.dma_start(w0s[:], w0_d[:])
        whs = {}
        for li in range(1, 8):
            whs[li] = ctile(f"whs{li}", [H, H], F16)
            nc.sync.dma_start(whs[li][:], wh_d[li][:])
        bss = {}
        for li in range(0, 8):
            bss[li] = ctile(f"bs{li}", [H, 1], F32)
            nc.sync.dma_start(bss[li][:], b_d[li][:])
        c0s = ctile("c0s", [H, 12], F32)
        nc.sync.dma_start(c0s[:], c0_d[:])
        w8cs = ctile("w8cs", [H, 16 * 13], F16)
        nc.sync.dma_start(w8cs[:], w8c_d[:])
        b8s = ctile("b8s", [H, 1], F32)
        nc.sync.dma_start(b8s[:], b8_d[:])
        lams = ctile("lams", [H, 4], F32)
        nc.sync.dma_start(lams[:], lam_d[:])

        z8stage = misc.tile([16, NLOC], F32, name="z8stage", tag="z8stage")

        V, G, S = nc.vector, nc.gpsimd, nc.scalar

        def new_A(streams_adds):
            A = {}
            for s, na in streams_adds.items():
                A[s] = [apool.tile([H, BLK], F16, name=f"A_{s}_{j}",
                                   tag=f"A_{s}_{j}") for j in range(na)]
            return A

        def alloc_chain():
            s_t = chain.tile([H, BLK], F32, name="s_t", tag="s_t")
            t1 = chain.tile([H, BLK], F32, name="t1", tag="t1")
            s1 = chain.tile([H, BLK], F16, name="s1", tag="s1", bufs=2)
            w3 = chain.tile([H, BLK], F16, name="w3", tag="w3")
            s2m = chain.tile([H, BLK], F16, name="s2m", tag="s2m")
            s3h = chain.tile([H, BLK], F16, name="s3h", tag="s3h")
            vhi2 = chain.tile([H, BLK], F16, name="vhi2", tag="vhi2")
            return s_t, t1, s1, w3, s2m, s3h, vhi2

        def chunk_chain(li, zt, s_t, t1, s1, csl):
            # tanh(zv + b) and the per-chunk pieces needed to release PSUM
            S.activation(s_t[:, csl], zt[:], AF.Tanh, bias=bss[li][:])
            S.activation(t1[:, csl], s_t[:, csl], AF.Square)
            S.activation(s1[:, csl], t1[:, csl], AF.Copy, bias=1.0, scale=-1.0)

        def batch_chain(A_new, s_t, t1, s1, w3, s2m, s3h, vhi2):
            S.activation(A_new["v"][0][:], s_t[:], AF.Copy)       # vhi f16
            S.activation(vhi2[:], s_t[:], AF.Copy, scale=-2.0)
            S.activation(w3[:], t1[:], AF.Copy, bias=-2.0, scale=6.0)
            V.tensor_tensor(s2m[:], vhi2[:], s1[:], OP.mult)      # s''
            V.tensor_tensor(s3h[:], w3[:], s1[:], OP.mult)        # s'''

        # ---------------- layer emitters ----------------
        def layer0(blk):
            ptsb = misc.tile([3, BLK], F32, name="ptsb", tag="ptsb", bufs=2)
            nc.sync.dma_start(ptsb[:], pts_d[:, bass.ts(blk, BLK)])
            A_new = new_A({s: 1 for s in STREAMS})
            s_t, t1, s1, w3, s2m, s3h, vhi2 = alloc_chain()
            for c in range(NCH):
                csl = bass.ts(c, CH)
                zt = zpool.tile([H, CH], F32, name="z0", tag="z")
                nc.tensor.matmul(zt[:], w0s[:], ptsb[:, csl],
                                 start=True, stop=True)
                chunk_chain(0, zt, s_t, t1, s1, csl)
            batch_chain(A_new, s_t, t1, s1, w3, s2m, s3h, vhi2)
            # per-unit constant scales: col 0..11 =
            # cx cy ct gxx gxy gyy gxt gyt txxx txxy txyy tyyy
            src = {"x": (s1, 0), "y": (s1, 1), "t": (s1, 2),
                   "xx": (s2m, 3), "xy": (s2m, 4), "yy": (s2m, 5),
                   "xt": (s2m, 6), "yt": (s2m, 7),
                   "xxx": (s3h, 8), "xxy": (s3h, 9),
                   "xyy": (s3h, 10), "yyy": (s3h, 11)}
            for s, (t, col) in src.items():
                V.tensor_scalar(A_new[s][0][:], t[:], c0s[:, col:col + 1],
                                None, OP.mult)
            return A_new

        def hidden_layer(li, A_prev):
            A_new = new_A(NADD)
            s_t, t1, s1, w3, s2m, s3h, vhi2 = alloc_chain()
            zc = {s: zcp.tile([H, BLK], F16, name=f"zc_{s}", tag=f"zc_{s}")
                  for s in ZCOPY}
            for c in range(NCH):
                csl = bass.ts(c, CH)
                for s in STREAMS:
                    zt = zpool.tile([H, CH], F32, name=f"z_{s}", tag="z")
                    adds = A_prev[s]
                    for j, a in enumerate(adds):
                        nc.tensor.matmul(zt[:], whs[li][:], a[:, csl],
                                         start=(j == 0),
                                         stop=(j == len(adds) - 1))
                    if s == "v":
                        chunk_chain(li, zt, s_t, t1, s1, csl)
                    elif s in ZCOPY:
                        S.activation(zc[s][:, csl], zt[:], AF.Copy)
                    else:
                        j = PSUM_DIRECT[s]
                        V.tensor_tensor(A_new[s][j][:, csl], s1[:, csl],
                                        zt[:], OP.mult)
            batch_chain(A_new, s_t, t1, s1, w3, s2m, s3h, vhi2)
            pxx = misc.tile([H, BLK], F16, name="pxx", tag="pxx")
            pyy = misc.tile([H, BLK], F16, name="pyy", tag="pyy")
            ex = misc.tile([H, BLK], F16, name="ex", tag="ex")
            ey = misc.tile([H, BLK], F16, name="ey", tag="ey")
            fx = misc.tile([H, BLK], F16, name="fx", tag="fx")
            fy = misc.tile([H, BLK], F16, name="fy", tag="fy")
            S.activation(pxx[:], zc["x"][:], AF.Square)
            S.activation(pyy[:], zc["y"][:], AF.Square)
            tmp = [misc.tile([H, BLK], F16, name=f"tmp{i}", tag=f"tmp{i}")
                   for i in range(5)]
            # ex = s''*z_x = -2*E_x ; fx = s'''*z_x^2 = 2*F_x
            V.tensor_tensor(ex[:], s2m[:], zc["x"][:], OP.mult)
            V.tensor_tensor(ey[:], s2m[:], zc["y"][:], OP.mult)
            V.tensor_tensor(fx[:], s3h[:], pxx[:], OP.mult)
            V.tensor_tensor(fy[:], s3h[:], pyy[:], OP.mult)
            G.tensor_tensor(A_new["x"][0][:], s1[:], zc["x"][:], OP.mult)
            G.tensor_tensor(A_new["y"][0][:], s1[:], zc["y"][:], OP.mult)
            G.tensor_tensor(A_new["t"][0][:], s1[:], zc["t"][:], OP.mult)
            # merged second-deriv streams: A = ex*zc_a + s1*zc_aa
            G.tensor_tensor(A_new["xx"][0][:], ex[:], zc["x"][:], OP.mult)
            V.tensor_tensor(tmp[0][:], s1[:], zc["xx"][:], OP.mult)
            G.tensor_tensor(A_new["xx"][0][:], A_new["xx"][0][:], tmp[0][:],
                            OP.add)
            G.tensor_tensor(A_new["xy"][0][:], ex[:], zc["y"][:], OP.mult)
            V.tensor_tensor(tmp[1][:], s1[:], zc["xy"][:], OP.mult)
            G.tensor_tensor(A_new["xy"][0][:], A_new["xy"][0][:], tmp[1][:],
                            OP.add)
            G.tensor_tensor(A_new["yy"][0][:], ey[:], zc["y"][:], OP.mult)
            V.tensor_tensor(tmp[2][:], s1[:], zc["yy"][:], OP.mult)
            G.tensor_tensor(A_new["yy"][0][:], A_new["yy"][0][:], tmp[2][:],
                            OP.add)
            # xt/yt: chunk loop wrote s1*z into A[0]; add ex*zc_t / ey*zc_t
            V.tensor_tensor(tmp[3][:], ex[:], zc["t"][:], OP.mult)
            G.tensor_tensor(A_new["xt"][0][:], A_new["xt"][0][:], tmp[3][:],
                            OP.add)
            V.tensor_tensor(tmp[4][:], ey[:], zc["t"][:], OP.mult)
            G.tensor_tensor(A_new["yt"][0][:], A_new["yt"][0][:], tmp[4][:],
                            OP.add)
            # third-deriv T terms
            V.tensor_tensor(A_new["xxx"][0][:], fx[:], zc["x"][:], OP.mult)
            V.scalar_tensor_tensor(A_new["xxx"][1][:], ex[:], 3.0,
                                   zc["xx"][:], OP.mult, OP.mult)
            V.tensor_tensor(A_new["xxy"][0][:], fx[:], zc["y"][:], OP.mult)
            V.tensor_tensor(A_new["xxy"][1][:], ey[:], zc["xx"][:], OP.mult)
            V.scalar_tensor_tensor(A_new["xxy"][2][:], ex[:], 2.0,
                                   zc["xy"][:], OP.mult, OP.mult)
            V.tensor_tensor(A_new["xyy"][0][:], fy[:], zc["x"][:], OP.mult)
            V.tensor_tensor(A_new["xyy"][1][:], ex[:], zc["yy"][:], OP.mult)
            V.scalar_tensor_tensor(A_new["xyy"][2][:], ey[:], 2.0,
                                   zc["xy"][:], OP.mult, OP.mult)
            V.tensor_tensor(A_new["yyy"][0][:], fy[:], zc["y"][:], OP.mult)
            V.scalar_tensor_tensor(A_new["yyy"][1][:], ey[:], 3.0,
                                   zc["yy"][:], OP.mult, OP.mult)
            return A_new

        def layer8(blk, A_prev):
            total = sum(len(v) for v in A_prev.values())
            for c in range(NCH):
                csl = bass.ts(c, CH)
                z8 = z8pool.tile([16, CH], F32, name="z8", tag="z8")
                k = 0
                for si, s in enumerate(STREAMS):
                    w8blk = w8cs[:, 16 * si:16 * si + 16]
                    for a in A_prev[s]:
                        nc.tensor.matmul(z8[:], w8blk, a[:, csl],
                                         start=(k == 0), stop=(k == total - 1))
                        k += 1
                S.activation(z8stage[:, bass.ts(blk * NCH + c, CH)],
                             z8[:], AF.Copy)

        # ---------------- main loop ----------------
        for blk in range(NBLK):
            A = layer0(blk)
            for li in range(1, 8):
                A = hidden_layer(li, A)
            layer8(blk, A)

        # ---------------- final fp32 jet -> outputs ----------------
        def ft(name):
            return fpool.tile([H, PB], F32, name=name, tag=name)

        Z = {}
        for si, s in enumerate(STREAMS):
            Z[s] = ft(f"Z_{s}")
            nc.sync.dma_start(Z[s][:], z8stage[si:si + 1, :])

        def tt(name, a, b, op=OP.mult):
            o = ft(name)
            V.tensor_tensor(o[:], a[:], b[:], op)
            return o

        def stt(name, a, sc, b, op0=OP.mult, op1=OP.mult):
            o = ft(name)
            V.scalar_tensor_tensor(o[:], a[:], sc, b[:], op0, op1)
            return o

        s8 = ft("s8")
        S.activation(s8[:], Z["v"][:], AF.Tanh, bias=b8s[:])
        t18 = ft("t18")
        S.activation(t18[:], s8[:], AF.Square)
        s18 = ft("s18")
        S.activation(s18[:], t18[:], AF.Copy, bias=1.0, scale=-1.0)
        w38 = ft("w38")
        S.activation(w38[:], t18[:], AF.Copy, bias=-1.0, scale=3.0)
        s2m8 = tt("s2m8", s8, s18)            # s2 = -2*s2m8
        s3h8 = tt("s3h8", w38, s18)           # s3 = 2*s3h8
        e8x = tt("e8x", s2m8, Z["x"])
        e8y = tt("e8y", s2m8, Z["y"])
        p8xx = ft("p8xx")
        S.activation(p8xx[:], Z["x"][:], AF.Square)
        p8yy = ft("p8yy")
        S.activation(p8yy[:], Z["y"][:], AF.Square)
        f8x = tt("f8x", s3h8, p8xx)
        f8y = tt("f8y", s3h8, p8yy)

        u = tt("u", s18, Z["y"])                      # u = p_y
        vv = stt("vv", s18, -1.0, Z["x"])             # v = -p_x

        def second(name, Ea, Zb, Zdd):
            a1 = stt(name + "_a", Ea, -2.0, Zb)
            a2 = tt(name + "_b", s18, Zdd)
            return tt(name, a1, a2, OP.add)

        p_xx = second("p_xx", e8x, Z["x"], Z["xx"])
        p_xy = second("p_xy", e8x, Z["y"], Z["xy"])
        p_yy = second("p_yy", e8y, Z["y"], Z["yy"])
        p_yt = second("p_yt", e8y, Z["t"], Z["yt"])
        mp_xt_a = stt("mp_xt_a", e8x, 2.0, Z["t"])
        mp_xt_b = stt("mp_xt_b", s18, -1.0, Z["xt"])
        mp_xt = tt("mp_xt", mp_xt_a, mp_xt_b, OP.add)  # -p_xt

        def third3(name, Fa, Za, Ea, Zaa, Zddd):
            a1 = stt(name + "_a", Fa, 2.0, Za)
            a2 = stt(name + "_b", Ea, -6.0, Zaa)
            a3 = tt(name + "_c", s18, Zddd)
            a12 = tt(name + "_ab", a1, a2, OP.add)
            return tt(name, a12, a3, OP.add)

        p_xxx = third3("p_xxx", f8x, Z["x"], e8x, Z["xx"], Z["xxx"])
        p_yyy = third3("p_yyy", f8y, Z["y"], e8y, Z["yy"], Z["yyy"])

        def third_m(name, Fa, Zb, Eb, Zaa, Ea, Zab, Zddd):
            # 2*Fa*Zb - 2*Eb*Zaa - 4*Ea*Zab + s1*Zddd
            a1 = stt(name + "_a", Fa, 2.0, Zb)
            a2 = stt(name + "_b", Eb, -2.0, Zaa)
            a3 = stt(name + "_c", Ea, -4.0, Zab)
            a4 = tt(name + "_d", s18, Zddd)
            a12 = tt(name + "_ab", a1, a2, OP.add)
            a34 = tt(name + "_cd", a3, a4, OP.add)
            return tt(name, a12, a34, OP.add)

        p_xxy = third_m("p_xxy", f8x, Z["y"], e8y, Z["xx"], e8x, Z["xy"],
                        Z["xxy"])
        p_xyy = third_m("p_xyy", f8y, Z["x"], e8x, Z["yy"], e8y, Z["xy"],
                        Z["xyy"])

        # f_u = p_yt + lam1*(u*p_xy + v*p_yy) - lam2*(p_xxy + p_yyy)
        fu_a = tt("fu_a", u, p_xy)
        fu_b = tt("fu_b", vv, p_yy)
        fu_ab = tt("fu_ab", fu_a, fu_b, OP.add)
        fu_l = stt("fu_l", fu_ab, lams[:, 0:1], p_yt, OP.mult, OP.add)
        fu_c = tt("fu_c", p_xxy, p_yyy, OP.add)
        f_u = stt("f_u", fu_c, lams[:, 1:2], fu_l, OP.mult, OP.add)
        # f_v = -p_xt - lam1*(u*p_xx + v*p_xy) + lam2*(p_xxx + p_xyy)
        fv_a = tt("fv_a", u, p_xx)
        fv_b = tt("fv_b", vv, p_xy)
        fv_ab = tt("fv_ab", fv_a, fv_b, OP.add)
        fv_l = stt("fv_l", fv_ab, lams[:, 2:3], mp_xt, OP.mult, OP.add)
        fv_c = tt("fv_c", p_xxx, p_xyy, OP.add)
        f_v = stt("f_v", fv_c, lams[:, 3:4], fv_l, OP.mult, OP.add)

        nc.sync.dma_start(out_d["uo"][:], u[:])
        nc.sync.dma_start(out_d["vo"][:], vv[:])
        nc.sync.dma_start(out_d["fuo"][:], f_u[:])
        nc.sync.dma_start(out_d["fvo"][:], f_v[:])

    return nc


_CACHE = {}


def _get_nc():
    if "nc" not in _CACHE:
        nc = _build()
        nc.finalize()
        _CACHE["nc"] = nc
    return _CACHE["nc"]


def kernel(**inputs):
    nc = _get_nc()
    f32 = np.float32
    x = np.asarray(inputs["x"], f32)[:, 0]
    y = np.asarray(inputs["y"], f32)[:, 0]
    t = np.asarray(inputs["t"], f32)[:, 0]
    pts = np.ascontiguousarray(np.stack([x, y, t], 0))          # [3, N]
    W0 = np.asarray(inputs["W0"], f32)
    cx, cy, ct = W0[0], W0[1], W0[2]
    c0 = np.ascontiguousarray(np.stack(
        [cx, cy, ct,
         cx * cx, cx * cy, cy * cy, cx * ct, cy * ct,
         cx ** 3, cx * cx * cy, cx * cy * cy, cy ** 3],
        1).astype(f32))                                         # [128, 12]
    w8 = np.asarray(inputs["W8"], f32)[:, 0].astype(np.float16)
    W8C = np.zeros([H, 16 * 13], np.float16)
    for s in range(13):
        W8C[:, 16 * s + s] = w8
    lam1 = f32(np.asarray(inputs["lam1"]).reshape(-1)[0])
    lam2 = f32(np.asarray(inputs["lam2"]).reshape(-1)[0])
    shared = {
        "W0f": np.ascontiguousarray(W0),
        "c0": c0,
        "W8C": W8C,
        "b8v": np.full([H, 1], np.asarray(inputs["b8"]).reshape(-1)[0], f32),
        "lam": np.tile(np.array([[lam1, -lam2, -lam1, lam2]], f32), (H, 1)),
    }
    for li in range(1, 8):
        shared[f"Wh{li}"] = np.asarray(inputs[f"W{li}"], f32).astype(
            np.float16)
    for li in range(0, 8):
        shared[f"bb{li}"] = np.asarray(
            inputs[f"b{li}"], f32).reshape(H, 1).copy()

    in_maps = []
    for c in range(N_CORES):
        m = dict(shared)
        m["pts"] = np.ascontiguousarray(pts[:, c * NLOC:(c + 1) * NLOC])
        in_maps.append(m)

    trace = bool(os.environ.get("BASS_KERNEL_TRACE"))
    tdir = os.environ.get("BASS_KERNEL_TRACE_DIR") or None
    res = run_bass_kernel_spmd(nc, in_maps, list(range(N_CORES)),
                               trace=trace, tmpdir=tdir)
    kernel.last_exec_time_ns = res.exec_time_ns
    outs = []
    for name in ["uo", "vo", "fuo", "fvo"]:
        full = np.concatenate(
            [np.asarray(res.results[c][name], f32).reshape(-1)
             for c in range(N_CORES)])
        outs.append(full[:, None])
    return tuple(outs)


kernel.last_exec_time_ns = None


# revision 17
# speedup vs baseline: 1.2507x; 1.2507x over previous
"""PINN Navier-Stokes residual kernel for trn2 (8 cores, data parallel).

Strategy: propagate a 13-stream first/second/third-order Taylor jet
{v,x,y,t,xx,xy,yy,xt,yt,xxx,xxy,xyy,yyy} through the 3-128x8-2 tanh MLP.
Activations live as [128 hidden, Npts free] tiles; each hidden layer is
28 fp16 matmuls per 512-pt chunk (addends of each stream kept separate and
accumulated in PSUM), followed by fp16 elementwise chain-rule updates split
across Vector/GpSimd/Scalar engines.  Layer 0 and the final jet->(u,v,f_u,f_v)
stage are fp32.
"""

import os
import numpy as np
from contextlib import ExitStack

import concourse.bass as bass
import concourse.bacc as bacc
import concourse.tile as tile
from concourse import mybir
from concourse.bass_utils import run_bass_kernel_spmd

F32 = mybir.dt.float32
F16 = mybir.dt.float16
OP = mybir.AluOpType
AF = mybir.ActivationFunctionType

N_CORES = 8
N = 32768
NLOC = N // N_CORES      # 4096 points per core
BLK = 1024               # elementwise batch
NBLK = NLOC // BLK       # 4
CH = 512                 # matmul / psum chunk
NCH = BLK // CH          # 2
H = 128
PB = NLOC // H           # 32 free cols in final [128, PB] tiles

STREAMS = ["v", "x", "y", "t", "xx", "xy", "yy", "xt", "yt",
           "xxx", "xxy", "xyy", "yyy"]
# addend counts for A produced by hidden layers 1..7
NADD = {"v": 1, "x": 1, "y": 1, "t": 1, "xx": 2, "xy": 2, "yy": 2,
        "xt": 2, "yt": 2, "xxx": 3, "xxy": 4, "xyy": 4, "yyy": 3}
ZCOPY = ("x", "y", "t", "xx", "xy", "yy")
# streams whose z is consumed directly from PSUM (-> addend index)
PSUM_DIRECT = {"xt": 1, "yt": 1, "xxx": 2, "xxy": 3, "xyy": 3, "yyy": 2}


def _build():
    nc = bacc.Bacc(None, target_bir_lowering=False)

    pts_d = nc.declare_dram_parameter("pts", [3, NLOC], F32, False)
    w0_d = nc.declare_dram_parameter("W0f", [3, H], F32, False)
    wh_d = {li: nc.declare_dram_parameter(f"Wh{li}", [H, H], F16, False)
            for li in range(1, 8)}
    b_d = {li: nc.declare_dram_parameter(f"bb{li}", [H, 1], F32, False)
           for li in range(0, 8)}
    c0_d = nc.declare_dram_parameter("c0", [H, 12], F32, False)
    w8c_d = nc.declare_dram_parameter("W8C", [H, 16 * 13], F16, False)
    b8_d = nc.declare_dram_parameter("b8v", [H, 1], F32, False)
    lam_d = nc.declare_dram_parameter("lam", [H, 4], F32, False)
    out_d = {k: nc.declare_dram_parameter(k, [H, PB], F32, True)
             for k in ["uo", "vo", "fuo", "fvo"]}

    with tile.TileContext(nc) as tc, ExitStack() as ctx:
        cpool = ctx.enter_context(tc.tile_pool(name="consts", bufs=1))
        apool = ctx.enter_context(tc.tile_pool(name="A", bufs=2))
        chain = ctx.enter_context(tc.tile_pool(name="chain", bufs=1))
        zcp = ctx.enter_context(tc.tile_pool(name="zc", bufs=1))
        misc = ctx.enter_context(tc.tile_pool(name="misc", bufs=1))
        fpool = ctx.enter_context(tc.tile_pool(name="fin", bufs=1))
        zpool = ctx.enter_context(
            tc.tile_pool(name="psum_z", bufs=7, space="PSUM"))
        z8pool = ctx.enter_context(
            tc.tile_pool(name="psum_z8", bufs=1, space="PSUM"))

        def ctile(name, shape, dt):
            t = cpool.tile(shape, dt, name=name, tag=name)
            return t

        w0s = ctile("w0s", [3, H], F32)
        nc.sync.dma_start(w0s[:], w0_d[:])
        whs = {}
        for li in range(1, 8):
            whs[li] = ctile(f"whs{li}", [H, H], F16)
            nc.sync.dma_start(whs[li][:], wh_d[li][:])
        bss = {}
        for li in range(0, 8):
            bss[li] = ctile(f"bs{li}", [H, 1], F32)
            nc.sync.dma_start(bss[li][:], b_d[li][:])
        c0s = ctile("c0s", [H, 12], F32)
        nc.sync.dma_start(c0s[:], c0_d[:])
        w8cs = ctile("w8cs", [H, 16 * 13], F16)
        nc.sync.dma_start(w8cs[:], w8c_d[:])
        b8s = ctile("b8s", [H, 1], F32)
        nc.sync.dma_start(b8s[:], b8_d[:])
        lams = ctile("lams", [H, 4], F32)
        nc.sync.dma_start(lams[:], lam_d[:])

        z8stage = misc.tile([16, NLOC], F32, name="z8stage", tag="z8stage")

        V, G, S = nc.vector, nc.gpsimd, nc.scalar

        def new_A(streams_adds):
            A = {}
            for s, na in streams_adds.items():
                A[s] = [apool.tile([H, BLK], F16, name=f"A_{s}_{j}",
                                   tag=f"A_{s}_{j}") for j in range(na)]
            return A

        def alloc_chain():
            s_t = chain.tile([H, BLK], F32, name="s_t", tag="s_t")
            t1 = chain.tile([H, BLK], F32, name="t1", tag="t1")
            s1 = chain.tile([H, BLK], F16, name="s1", tag="s1", bufs=2)
            w3 = chain.tile([H, BLK], F16, name="w3", tag="w3")
            s2m = chain.tile([H, BLK], F16, name="s2m", tag="s2m")
            s3h = chain.tile([H, BLK], F16, name="s3h", tag="s3h")
            vhi2 = chain.tile([H, BLK], F16, name="vhi2", tag="vhi2")
            return s_t, t1, s1, w3, s2m, s3h, vhi2

        def chunk_chain(li, zt, s_t, t1, s1, csl):
            # tanh(zv + b) and the per-chunk pieces needed to release PSUM
            S.activation(s_t[:, csl], zt[:], AF.Tanh, bias=bss[li][:])
            S.activation(t1[:, csl], s_t[:, csl], AF.Square)
            S.activation(s1[:, csl], t1[:, csl], AF.Copy, bias=1.0, scale=-1.0)

        def batch_chain(A_new, s_t, t1, s1, w3, s2m, s3h, vhi2):
            S.activation(A_new["v"][0][:], s_t[:], AF.Copy)       # vhi f16
            S.activation(vhi2[:], s_t[:], AF.Copy, scale=-2.0)
            S.activation(w3[:], t1[:], AF.Copy, bias=-2.0, scale=6.0)
            V.tensor_tensor(s2m[:], vhi2[:], s1[:], OP.mult)      # s''
            V.tensor_tensor(s3h[:], w3[:], s1[:], OP.mult)        # s'''

        # ---------------- layer emitters ----------------
        def layer0(blk):
            ptsb = misc.tile([3, BLK], F32, name="ptsb", tag="ptsb", bufs=2)
            nc.sync.dma_start(ptsb[:], pts_d[:, bass.ts(blk, BLK)])
            A_new = new_A({s: 1 for s in STREAMS})
            s_t, t1, s1, w3, s2m, s3h, vhi2 = alloc_chain()
            for c in range(NCH):
                csl = bass.ts(c, CH)
                zt = zpool.tile([H, CH], F32, name="z0", tag="z")
                nc.tensor.matmul(zt[:], w0s[:], ptsb[:, csl],
                                 start=True, stop=True)
                chunk_chain(0, zt, s_t, t1, s1, csl)
            batch_chain(A_new, s_t, t1, s1, w3, s2m, s3h, vhi2)
            # per-unit constant scales: col 0..11 =
            # cx cy ct gxx gxy gyy gxt gyt txxx txxy txyy tyyy
            src = {"x": (s1, 0), "y": (s1, 1), "t": (s1, 2),
                   "xx": (s2m, 3), "xy": (s2m, 4), "yy": (s2m, 5),
                   "xt": (s2m, 6), "yt": (s2m, 7),
                   "xxx": (s3h, 8), "xxy": (s3h, 9),
                   "xyy": (s3h, 10), "yyy": (s3h, 11)}
            for s, (t, col) in src.items():
                V.tensor_scalar(A_new[s][0][:], t[:], c0s[:, col:col + 1],
                                None, OP.mult)
            return A_new

        def hidden_layer(li, A_prev):
            A_new = new_A(NADD)
            s_t, t1, s1, w3, s2m, s3h, vhi2 = alloc_chain()
            zc = {s: zcp.tile([H, BLK], F16, name=f"zc_{s}", tag=f"zc_{s}")
                  for s in ZCOPY}
            for c in range(NCH):
                csl = bass.ts(c, CH)
                for s in STREAMS:
                    zt = zpool.tile([H, CH], F32, name=f"z_{s}", tag="z")
                    adds = A_prev[s]
                    for j, a in enumerate(adds):
                        nc.tensor.matmul(zt[:], whs[li][:], a[:, csl],
                                         start=(j == 0),
                                         stop=(j == len(adds) - 1))
                    if s == "v":
                        chunk_chain(li, zt, s_t, t1, s1, csl)
                    elif s in ZCOPY:
                        S.activation(zc[s][:, csl], zt[:], AF.Copy)
                    else:
                        j = PSUM_DIRECT[s]
                        V.tensor_tensor(A_new[s][j][:, csl], s1[:, csl],
                                        zt[:], OP.mult)
            batch_chain(A_new, s_t, t1, s1, w3, s2m, s3h, vhi2)
            pxx = misc.tile([H, BLK], F16, name="pxx", tag="pxx")
            pyy = misc.tile([H, BLK], F16, name="pyy", tag="pyy")
            ex = misc.tile([H, BLK], F16, name="ex", tag="ex")
            ey = misc.tile([H, BLK], F16, name="ey", tag="ey")
            fx = misc.tile([H, BLK], F16, name="fx", tag="fx")
            fy = misc.tile([H, BLK], F16, name="fy", tag="fy")
            S.activation(pxx[:], zc["x"][:], AF.Square)
            S.activation(pyy[:], zc["y"][:], AF.Square)
            # ex = s''*z_x = -2*E_x ; fx = s'''*z_x^2 = 2*F_x
            V.tensor_tensor(ex[:], s2m[:], zc["x"][:], OP.mult)
            V.tensor_tensor(ey[:], s2m[:], zc["y"][:], OP.mult)
            V.tensor_tensor(fx[:], s3h[:], pxx[:], OP.mult)
            V.tensor_tensor(fy[:], s3h[:], pyy[:], OP.mult)
            G.tensor_tensor(A_new["x"][0][:], s1[:], zc["x"][:], OP.mult)
            G.tensor_tensor(A_new["y"][0][:], s1[:], zc["y"][:], OP.mult)
            G.tensor_tensor(A_new["t"][0][:], s1[:], zc["t"][:], OP.mult)
            # G terms (second derivs) and H terms
            V.tensor_tensor(A_new["xx"][0][:], ex[:], zc["x"][:], OP.mult)
            V.tensor_tensor(A_new["xy"][0][:], ex[:], zc["y"][:], OP.mult)
            V.tensor_tensor(A_new["yy"][0][:], ey[:], zc["y"][:], OP.mult)
            V.tensor_tensor(A_new["xt"][0][:], ex[:], zc["t"][:], OP.mult)
            V.tensor_tensor(A_new["yt"][0][:], ey[:], zc["t"][:], OP.mult)
            G.tensor_tensor(A_new["xx"][1][:], s1[:], zc["xx"][:], OP.mult)
            G.tensor_tensor(A_new["xy"][1][:], s1[:], zc["xy"][:], OP.mult)
            G.tensor_tensor(A_new["yy"][1][:], s1[:], zc["yy"][:], OP.mult)
            # third-deriv T terms
            V.tensor_tensor(A_new["xxx"][0][:], fx[:], zc["x"][:], OP.mult)
            V.scalar_tensor_tensor(A_new["xxx"][1][:], ex[:], 3.0,
                                   zc["xx"][:], OP.mult, OP.mult)
            V.tensor_tensor(A_new["xxy"][0][:], fx[:], zc["y"][:], OP.mult)
            V.tensor_tensor(A_new["xxy"][1][:], ey[:], zc["xx"][:], OP.mult)
            V.scalar_tensor_tensor(A_new["xxy"][2][:], ex[:], 2.0,
                                   zc["xy"][:], OP.mult, OP.mult)
            V.tensor_tensor(A_new["xyy"][0][:], fy[:], zc["x"][:], OP.mult)
            V.tensor_tensor(A_new["xyy"][1][:], ex[:], zc["yy"][:], OP.mult)
            V.scalar_tensor_tensor(A_new["xyy"][2][:], ey[:], 2.0,
                                   zc["xy"][:], OP.mult, OP.mult)
            V.tensor_tensor(A_new["yyy"][0][:], fy[:], zc["y"][:], OP.mult)
            V.scalar_tensor_tensor(A_new["yyy"][1][:], ey[:], 3.0,
                                   zc["yy"][:], OP.mult, OP.mult)
            return A_new

        def layer8(blk, A_prev):
            total = sum(len(v) for v in A_prev.values())
            for c in range(NCH):
                csl = bass.ts(c, CH)
                z8 = z8pool.tile([16, CH], F32, name="z8", tag="z8")
                k = 0
                for si, s in enumerate(STREAMS):
                    w8blk = w8cs[:, 16 * si:16 * si + 16]
                    for a in A_prev[s]:
                        nc.tensor.matmul(z8[:], w8blk, a[:, csl],
                                         start=(k == 0), stop=(k == total - 1))
                        k += 1
                S.activation(z8stage[:, bass.ts(blk * NCH + c, CH)],
                             z8[:], AF.Copy)

        # ---------------- main loop ----------------
        for blk in range(NBLK):
            A = layer0(blk)
            for li in range(1, 8):
                A = hidden_layer(li, A)
            layer8(blk, A)

        # ---------------- final fp32 jet -> outputs ----------------
        def ft(name):
            return fpool.tile([H, PB], F32, name=name, tag=name)

        Z = {}
        for si, s in enumerate(STREAMS):
            Z[s] = ft(f"Z_{s}")
            nc.sync.dma_start(Z[s][:], z8stage[si:si + 1, :])

        def tt(name, a, b, op=OP.mult):
            o = ft(name)
            V.tensor_tensor(o[:], a[:], b[:], op)
            return o

        def stt(name, a, sc, b, op0=OP.mult, op1=OP.mult):
            o = ft(name)
            V.scalar_tensor_tensor(o[:], a[:], sc, b[:], op0, op1)
            return o

        s8 = ft("s8")
        S.activation(s8[:], Z["v"][:], AF.Tanh, bias=b8s[:])
        t18 = ft("t18")
        S.activation(t18[:], s8[:], AF.Square)
        s18 = ft("s18")
        S.activation(s18[:], t18[:], AF.Copy, bias=1.0, scale=-1.0)
        w38 = ft("w38")
        S.activation(w38[:], t18[:], AF.Copy, bias=-1.0, scale=3.0)
        s2m8 = tt("s2m8", s8, s18)            # s2 = -2*s2m8
        s3h8 = tt("s3h8", w38, s18)           # s3 = 2*s3h8
        e8x = tt("e8x", s2m8, Z["x"])
        e8y = tt("e8y", s2m8, Z["y"])
        p8xx = ft("p8xx")
        S.activation(p8xx[:], Z["x"][:], AF.Square)
        p8yy = ft("p8yy")
        S.activation(p8yy[:], Z["y"][:], AF.Square)
        f8x = tt("f8x", s3h8, p8xx)
        f8y = tt("f8y", s3h8, p8yy)

        u = tt("u", s18, Z["y"])                      # u = p_y
        vv = stt("vv", s18, -1.0, Z["x"])             # v = -p_x

        def second(name, Ea, Zb, Zdd):
            a1 = stt(name + "_a", Ea, -2.0, Zb)
            a2 = tt(name + "_b", s18, Zdd)
            return tt(name, a1, a2, OP.add)

        p_xx = second("p_xx", e8x, Z["x"], Z["xx"])
        p_xy = second("p_xy", e8x, Z["y"], Z["xy"])
        p_yy = second("p_yy", e8y, Z["y"], Z["yy"])
        p_yt = second("p_yt", e8y, Z["t"], Z["yt"])
        mp_xt_a = stt("mp_xt_a", e8x, 2.0, Z["t"])
        mp_xt_b = stt("mp_xt_b", s18, -1.0, Z["xt"])
        mp_xt = tt("mp_xt", mp_xt_a, mp_xt_b, OP.add)  # -p_xt

        def third3(name, Fa, Za, Ea, Zaa, Zddd):
            a1 = stt(name + "_a", Fa, 2.0, Za)
            a2 = stt(name + "_b", Ea, -6.0, Zaa)
            a3 = tt(name + "_c", s18, Zddd)
            a12 = tt(name + "_ab", a1, a2, OP.add)
            return tt(name, a12, a3, OP.add)

        p_xxx = third3("p_xxx", f8x, Z["x"], e8x, Z["xx"], Z["xxx"])
        p_yyy = third3("p_yyy", f8y, Z["y"], e8y, Z["yy"], Z["yyy"])

        def third_m(name, Fa, Zb, Eb, Zaa, Ea, Zab, Zddd):
            # 2*Fa*Zb - 2*Eb*Zaa - 4*Ea*Zab + s1*Zddd
            a1 = stt(name + "_a", Fa, 2.0, Zb)
            a2 = stt(name + "_b", Eb, -2.0, Zaa)
            a3 = stt(name + "_c", Ea, -4.0, Zab)
            a4 = tt(name + "_d", s18, Zddd)
            a12 = tt(name + "_ab", a1, a2, OP.add)
            a34 = tt(name + "_cd", a3, a4, OP.add)
            return tt(name, a12, a34, OP.add)

        p_xxy = third_m("p_xxy", f8x, Z["y"], e8y, Z["xx"], e8x, Z["xy"],
                        Z["xxy"])
        p_xyy = third_m("p_xyy", f8y, Z["x"], e8x, Z["yy"], e8y, Z["xy"],
                        Z["xyy"])

        # f_u = p_yt + lam1*(u*p_xy + v*p_yy) - lam2*(p_xxy + p_yyy)
        fu_a = tt("fu_a", u, p_xy)
        fu_b = tt("fu_b", vv, p_yy)
        fu_ab = tt("fu_ab", fu_a, fu_b, OP.add)
        fu_l = stt("fu_l", fu_ab, lams[:, 0:1], p_yt, OP.mult, OP.add)
        fu_c = tt("fu_c", p_xxy, p_yyy, OP.add)
        f_u = stt("f_u", fu_c, lams[:, 1:2], fu_l, OP.mult, OP.add)
        # f_v = -p_xt - lam1*(u*p_xx + v*p_xy) + lam2*(p_xxx + p_xyy)
        fv_a = tt("fv_a", u, p_xx)
        fv_b = tt("fv_b", vv, p_xy)
        fv_ab = tt("fv_ab", fv_a, fv_b, OP.add)
        fv_l = stt("fv_l", fv_ab, lams[:, 2:3], mp_xt, OP.mult, OP.add)
        fv_c = tt("fv_c", p_xxx, p_xyy, OP.add)
        f_v = stt("f_v", fv_c, lams[:, 3:4], fv_l, OP.mult, OP.add)

        nc.sync.dma_start(out_d["uo"][:], u[:])
        nc.sync.dma_start(out_d["vo"][:], vv[:])
        nc.sync.dma_start(out_d["fuo"][:], f_u[:])
        nc.sync.dma_start(out_d["fvo"][:], f_v[:])

    return nc


_CACHE = {}


def _get_nc():
    if "nc" not in _CACHE:
        nc = _build()
        nc.finalize()
        _CACHE["nc"] = nc
    return _CACHE["nc"]


def kernel(**inputs):
    nc = _get_nc()
    f32 = np.float32
    x = np.asarray(inputs["x"], f32)[:, 0]
    y = np.asarray(inputs["y"], f32)[:, 0]
    t = np.asarray(inputs["t"], f32)[:, 0]
    pts = np.ascontiguousarray(np.stack([x, y, t], 0))          # [3, N]
    W0 = np.asarray(inputs["W0"], f32)
    cx, cy, ct = W0[0], W0[1], W0[2]
    c0 = np.ascontiguousarray(np.stack(
        [cx, cy, ct,
         cx * cx, cx * cy, cy * cy, cx * ct, cy * ct,
         cx ** 3, cx * cx * cy, cx * cy * cy, cy ** 3],
        1).astype(f32))                                         # [128, 12]
    w8 = np.asarray(inputs["W8"], f32)[:, 0].astype(np.float16)
    W8C = np.zeros([H, 16 * 13], np.float16)
    for s in range(13):
        W8C[:, 16 * s + s] = w8
    lam1 = f32(np.asarray(inputs["lam1"]).reshape(-1)[0])
    lam2 = f32(np.asarray(inputs["lam2"]).reshape(-1)[0])
    shared = {
        "W0f": np.ascontiguousarray(W0),
        "c0": c0,
        "W8C": W8C,
        "b8v": np.full([H, 1], np.asarray(inputs["b8"]).reshape(-1)[0], f32),
        "lam": np.tile(np.array([[lam1, -lam2, -lam1, lam2]], f32), (H, 1)),
    }
    for li in range(1, 8):
        shared[f"Wh{li}"] = np.asarray(inputs[f"W{li}"], f32).astype(
            np.float16)
    for li in range(0, 8):
        shared[f"bb{li}"] = np.asarray(
            inputs[f"b{li}"], f32).reshape(H, 1).copy()

    in_maps = []
    for c in range(N_CORES):
        m = dict(shared)
        m["pts"] = np.ascontiguousarray(pts[:, c * NLOC:(c + 1) * NLOC])
        in_maps.append(m)

    trace = bool(os.environ.get("BASS_KERNEL_TRACE"))
    tdir = os.environ.get("BASS_KERNEL_TRACE_DIR") or None
    res = run_bass_kernel_spmd(nc, in_maps, list(range(N_CORES)),
                               trace=trace, tmpdir=tdir)
    kernel.last_exec_time_ns = res.exec_time_ns
    outs = []
    for name in ["uo", "vo", "fuo", "fvo"]:
        full = np.concatenate(
            [np.asarray(res.results[c][name], f32).reshape(-1)
             for c in range(N_CORES)])
        outs.append(full[:, None])
    return tuple(outs)


kernel.last_exec_time_ns = None


# revision 20
# speedup vs baseline: 1.6072x; 1.2850x over previous
"""PINN Navier-Stokes residual kernel for trn2 (8 cores, data parallel).

Strategy: propagate a 13-stream first/second/third-order Taylor jet
{v,x,y,t,xx,xy,yy,xt,yt,xxx,xxy,xyy,yyy} through the 3-128x8-2 tanh MLP.
Activations live as [128 hidden, Npts free] tiles; each hidden layer is
28 fp16 matmuls per 512-pt chunk (addends of each stream kept separate and
accumulated in PSUM), followed by fp16 elementwise chain-rule updates split
across Vector/GpSimd/Scalar engines.  Layer 0 and the final jet->(u,v,f_u,f_v)
stage are fp32.
"""

import os
import numpy as np
from contextlib import ExitStack

import concourse.bass as bass
import concourse.bacc as bacc
import concourse.tile as tile
from concourse import mybir
from concourse.bass_utils import run_bass_kernel_spmd

F32 = mybir.dt.float32
F16 = mybir.dt.float16
OP = mybir.AluOpType
AF = mybir.ActivationFunctionType

N_CORES = 8
N = 32768
NLOC = N // N_CORES      # 4096 points per core
BLK = 1024               # elementwise batch
NBLK = NLOC // BLK       # 4
CH = 512                 # matmul / psum chunk
NCH = BLK // CH          # 2
H = 128
PB = NLOC // H           # 32 free cols in final [128, PB] tiles

STREAMS = ["v", "x", "y", "t", "xx", "xy", "yy", "xt", "yt",
           "xxx", "xxy", "xyy", "yyy"]
# addend counts for A produced by hidden layers 1..7
NADD = {"v": 1, "x": 1, "y": 1, "t": 1, "xx": 2, "xy": 2, "yy": 2,
        "xt": 2, "yt": 2, "xxx": 3, "xxy": 4, "xyy": 4, "yyy": 3}
ZCOPY = ("x", "y", "t", "xx", "xy", "yy")
# streams whose z is consumed directly from PSUM (-> addend index)
PSUM_DIRECT = {"xt": 1, "yt": 1, "xxx": 2, "xxy": 3, "xyy": 3, "yyy": 2}


def _build():
    nc = bacc.Bacc(None, target_bir_lowering=False)

    pts_d = nc.declare_dram_parameter("pts", [3, NLOC], F32, False)
    w0_d = nc.declare_dram_parameter("W0f", [3, H], F32, False)
    wh_d = {li: nc.declare_dram_parameter(f"Wh{li}", [H, H], F16, False)
            for li in range(1, 8)}
    b_d = {li: nc.declare_dram_parameter(f"bb{li}", [H, 1], F32, False)
           for li in range(0, 8)}
    c0_d = nc.declare_dram_parameter("c0", [H, 12], F32, False)
    w8c_d = nc.declare_dram_parameter("W8C", [H, 16 * 13], F16, False)
    b8_d = nc.declare_dram_parameter("b8v", [H, 1], F32, False)
    lam_d = nc.declare_dram_parameter("lam", [H, 4], F32, False)
    out_d = {k: nc.declare_dram_parameter(k, [H, PB], F32, True)
             for k in ["uo", "vo", "fuo", "fvo"]}

    with tile.TileContext(nc) as tc, ExitStack() as ctx:
        cpool = ctx.enter_context(tc.tile_pool(name="consts", bufs=1))
        apool = ctx.enter_context(tc.tile_pool(name="A", bufs=2))
        chain = ctx.enter_context(tc.tile_pool(name="chain", bufs=1))
        zcp = ctx.enter_context(tc.tile_pool(name="zc", bufs=1))
        misc = ctx.enter_context(tc.tile_pool(name="misc", bufs=1))
        fpool = ctx.enter_context(tc.tile_pool(name="fin", bufs=1))
        zpool = ctx.enter_context(
            tc.tile_pool(name="psum_z", bufs=7, space="PSUM"))
        z8pool = ctx.enter_context(
            tc.tile_pool(name="psum_z8", bufs=1, space="PSUM"))

        def ctile(name, shape, dt):
            t = cpool.tile(shape, dt, name=name, tag=name)
            return t

        w0s = ctile("w0s", [3, H], F32)
        nc.sync.dma_start(w0s[:], w0_d[:])
        whs = {}
        for li in range(1, 8):
            whs[li] = ctile(f"whs{li}", [H, H], F16)
            nc.sync.dma_start(whs[li][:], wh_d[li][:])
        bss = {}
        for li in range(0, 8):
            bss[li] = ctile(f"bs{li}", [H, 1], F32)
            nc.sync.dma_start(bss[li][:], b_d[li][:])
        c0s = ctile("c0s", [H, 12], F32)
        nc.sync.dma_start(c0s[:], c0_d[:])
        w8cs = ctile("w8cs", [H, 16 * 13], F16)
        nc.sync.dma_start(w8cs[:], w8c_d[:])
        b8s = ctile("b8s", [H, 1], F32)
        nc.sync.dma_start(b8s[:], b8_d[:])
        lams = ctile("lams", [H, 4], F32)
        nc.sync.dma_start(lams[:], lam_d[:])

        z8stage = misc.tile([16, NLOC], F32, name="z8stage", tag="z8stage")

        V, G, S = nc.vector, nc.gpsimd, nc.scalar

        def new_A(streams_adds):
            A = {}
            for s, na in streams_adds.items():
                A[s] = [apool.tile([H, BLK], F16, name=f"A_{s}_{j}",
                                   tag=f"A_{s}_{j}") for j in range(na)]
            return A

        def alloc_chain():
            s_t = chain.tile([H, BLK], F32, name="s_t", tag="s_t")
            t1 = chain.tile([H, BLK], F32, name="t1", tag="t1")
            s1 = chain.tile([H, BLK], F16, name="s1", tag="s1", bufs=2)
            w3 = chain.tile([H, BLK], F16, name="w3", tag="w3")
            s2m = chain.tile([H, BLK], F16, name="s2m", tag="s2m")
            s3h = chain.tile([H, BLK], F16, name="s3h", tag="s3h")
            vhi2 = chain.tile([H, BLK], F16, name="vhi2", tag="vhi2")
            return s_t, t1, s1, w3, s2m, s3h, vhi2

        def chunk_chain(li, zt, s_t, t1, s1, csl):
            # tanh(zv + b) and the per-chunk pieces needed to release PSUM
            S.activation(s_t[:, csl], zt[:], AF.Tanh, bias=bss[li][:])
            S.activation(t1[:, csl], s_t[:, csl], AF.Square)
            S.activation(s1[:, csl], t1[:, csl], AF.Copy, bias=1.0, scale=-1.0)

        def batch_chain(A_new, s_t, t1, s1, w3, s2m, s3h, vhi2):
            S.activation(A_new["v"][0][:], s_t[:], AF.Copy)       # vhi f16
            S.activation(vhi2[:], s_t[:], AF.Copy, scale=-2.0)
            S.activation(w3[:], t1[:], AF.Copy, bias=-2.0, scale=6.0)
            V.tensor_tensor(s2m[:], vhi2[:], s1[:], OP.mult)      # s''
            V.tensor_tensor(s3h[:], w3[:], s1[:], OP.mult)        # s'''

        # ---------------- layer emitters ----------------
        def layer0(blk):
            ptsb = misc.tile([3, BLK], F32, name="ptsb", tag="ptsb", bufs=2)
            nc.sync.dma_start(ptsb[:], pts_d[:, bass.ts(blk, BLK)])
            A_new = new_A({s: 1 for s in STREAMS})
            s_t, t1, s1, w3, s2m, s3h, vhi2 = alloc_chain()
            for c in range(NCH):
                csl = bass.ts(c, CH)
                zt = zpool.tile([H, CH], F32, name="z0", tag="z")
                nc.tensor.matmul(zt[:], w0s[:], ptsb[:, csl],
                                 start=True, stop=True)
                chunk_chain(0, zt, s_t, t1, s1, csl)
            batch_chain(A_new, s_t, t1, s1, w3, s2m, s3h, vhi2)
            # per-unit constant scales: col 0..11 =
            # cx cy ct gxx gxy gyy gxt gyt txxx txxy txyy tyyy
            src = {"x": (s1, 0), "y": (s1, 1), "t": (s1, 2),
                   "xx": (s2m, 3), "xy": (s2m, 4), "yy": (s2m, 5),
                   "xt": (s2m, 6), "yt": (s2m, 7),
                   "xxx": (s3h, 8), "xxy": (s3h, 9),
                   "xyy": (s3h, 10), "yyy": (s3h, 11)}
            for s, (t, col) in src.items():
                V.tensor_scalar(A_new[s][0][:], t[:], c0s[:, col:col + 1],
                                None, OP.mult)
            return A_new

        def hidden_layer(li, A_prev):
            A_new = new_A(NADD)
            s_t, t1, s1, w3, s2m, s3h, vhi2 = alloc_chain()
            zc = {s: zcp.tile([H, BLK], F16, name=f"zc_{s}", tag=f"zc_{s}")
                  for s in ZCOPY}
            for c in range(NCH):
                csl = bass.ts(c, CH)
                for s in STREAMS:
                    zt = zpool.tile([H, CH], F32, name=f"z_{s}", tag="z")
                    adds = A_prev[s]
                    for j, a in enumerate(adds):
                        nc.tensor.matmul(zt[:], whs[li][:], a[:, csl],
                                         start=(j == 0),
                                         stop=(j == len(adds) - 1))
                    if s == "v":
                        chunk_chain(li, zt, s_t, t1, s1, csl)
                    elif s in ZCOPY:
                        S.activation(zc[s][:, csl], zt[:], AF.Copy)
                    else:
                        j = PSUM_DIRECT[s]
                        V.tensor_tensor(A_new[s][j][:, csl], s1[:, csl],
                                        zt[:], OP.mult)
            batch_chain(A_new, s_t, t1, s1, w3, s2m, s3h, vhi2)
            pxx = misc.tile([H, BLK], F16, name="pxx", tag="pxx")
            pyy = misc.tile([H, BLK], F16, name="pyy", tag="pyy")
            ex = misc.tile([H, BLK], F16, name="ex", tag="ex")
            ey = misc.tile([H, BLK], F16, name="ey", tag="ey")
            fx = misc.tile([H, BLK], F16, name="fx", tag="fx")
            fy = misc.tile([H, BLK], F16, name="fy", tag="fy")
            S.activation(pxx[:], zc["x"][:], AF.Square)
            S.activation(pyy[:], zc["y"][:], AF.Square)
            # ex = s''*z_x = -2*E_x ; fx = s'''*z_x^2 = 2*F_x
            V.tensor_tensor(ex[:], s2m[:], zc["x"][:], OP.mult)
            V.tensor_tensor(ey[:], s2m[:], zc["y"][:], OP.mult)
            V.tensor_tensor(fx[:], s3h[:], pxx[:], OP.mult)
            V.tensor_tensor(fy[:], s3h[:], pyy[:], OP.mult)
            # scaled zc copies on Scalar (reuse pxx/pyy buffers once dead)
            zxx3 = misc.tile([H, BLK], F16, name="zxx3", tag="pxx")
            zyy3 = misc.tile([H, BLK], F16, name="zyy3", tag="pyy")
            zxy2 = misc.tile([H, BLK], F16, name="zxy2", tag="zxy2")
            S.activation(zxx3[:], zc["xx"][:], AF.Copy, scale=3.0)
            S.activation(zyy3[:], zc["yy"][:], AF.Copy, scale=3.0)
            S.activation(zxy2[:], zc["xy"][:], AF.Copy, scale=2.0)
            V.tensor_tensor(A_new["x"][0][:], s1[:], zc["x"][:], OP.mult)
            V.tensor_tensor(A_new["y"][0][:], s1[:], zc["y"][:], OP.mult)
            V.tensor_tensor(A_new["t"][0][:], s1[:], zc["t"][:], OP.mult)
            # G terms (second derivs) and H terms
            V.tensor_tensor(A_new["xx"][0][:], ex[:], zc["x"][:], OP.mult)
            V.tensor_tensor(A_new["xy"][0][:], ex[:], zc["y"][:], OP.mult)
            V.tensor_tensor(A_new["yy"][0][:], ey[:], zc["y"][:], OP.mult)
            V.tensor_tensor(A_new["xt"][0][:], ex[:], zc["t"][:], OP.mult)
            V.tensor_tensor(A_new["yt"][0][:], ey[:], zc["t"][:], OP.mult)
            V.tensor_tensor(A_new["xx"][1][:], s1[:], zc["xx"][:], OP.mult)
            V.tensor_tensor(A_new["xy"][1][:], s1[:], zc["xy"][:], OP.mult)
            V.tensor_tensor(A_new["yy"][1][:], s1[:], zc["yy"][:], OP.mult)
            # third-deriv T terms
            V.tensor_tensor(A_new["xxx"][0][:], fx[:], zc["x"][:], OP.mult)
            V.tensor_tensor(A_new["xxx"][1][:], ex[:], zxx3[:], OP.mult)
            V.tensor_tensor(A_new["xxy"][0][:], fx[:], zc["y"][:], OP.mult)
            V.tensor_tensor(A_new["xxy"][1][:], ey[:], zc["xx"][:], OP.mult)
            V.tensor_tensor(A_new["xxy"][2][:], ex[:], zxy2[:], OP.mult)
            V.tensor_tensor(A_new["xyy"][0][:], fy[:], zc["x"][:], OP.mult)
            V.tensor_tensor(A_new["xyy"][1][:], ex[:], zc["yy"][:], OP.mult)
            V.tensor_tensor(A_new["xyy"][2][:], ey[:], zxy2[:], OP.mult)
            V.tensor_tensor(A_new["yyy"][0][:], fy[:], zc["y"][:], OP.mult)
            V.tensor_tensor(A_new["yyy"][1][:], ey[:], zyy3[:], OP.mult)
            return A_new

        def layer8(blk, A_prev):
            total = sum(len(v) for v in A_prev.values())
            for c in range(NCH):
                csl = bass.ts(c, CH)
                z8 = z8pool.tile([16, CH], F32, name="z8", tag="z8")
                k = 0
                for si, s in enumerate(STREAMS):
                    w8blk = w8cs[:, 16 * si:16 * si + 16]
                    for a in A_prev[s]:
                        nc.tensor.matmul(z8[:], w8blk, a[:, csl],
                                         start=(k == 0), stop=(k == total - 1))
                        k += 1
                S.activation(z8stage[:, bass.ts(blk * NCH + c, CH)],
                             z8[:], AF.Copy)

        # ---------------- main loop ----------------
        for blk in range(NBLK):
            A = layer0(blk)
            for li in range(1, 8):
                A = hidden_layer(li, A)
            layer8(blk, A)

        # ---------------- final fp32 jet -> outputs ----------------
        def ft(name):
            return fpool.tile([H, PB], F32, name=name, tag=name)

        Z = {}
        for si, s in enumerate(STREAMS):
            Z[s] = ft(f"Z_{s}")
            nc.sync.dma_start(Z[s][:], z8stage[si:si + 1, :])

        def tt(name, a, b, op=OP.mult):
            o = ft(name)
            V.tensor_tensor(o[:], a[:], b[:], op)
            return o

        def stt(name, a, sc, b, op0=OP.mult, op1=OP.mult):
            o = ft(name)
            V.scalar_tensor_tensor(o[:], a[:], sc, b[:], op0, op1)
            return o

        s8 = ft("s8")
        S.activation(s8[:], Z["v"][:], AF.Tanh, bias=b8s[:])
        t18 = ft("t18")
        S.activation(t18[:], s8[:], AF.Square)
        s18 = ft("s18")
        S.activation(s18[:], t18[:], AF.Copy, bias=1.0, scale=-1.0)
        w38 = ft("w38")
        S.activation(w38[:], t18[:], AF.Copy, bias=-1.0, scale=3.0)
        s2m8 = tt("s2m8", s8, s18)            # s2 = -2*s2m8
        s3h8 = tt("s3h8", w38, s18)           # s3 = 2*s3h8
        e8x = tt("e8x", s2m8, Z["x"])
        e8y = tt("e8y", s2m8, Z["y"])
        p8xx = ft("p8xx")
        S.activation(p8xx[:], Z["x"][:], AF.Square)
        p8yy = ft("p8yy")
        S.activation(p8yy[:], Z["y"][:], AF.Square)
        f8x = tt("f8x", s3h8, p8xx)
        f8y = tt("f8y", s3h8, p8yy)

        u = tt("u", s18, Z["y"])                      # u = p_y
        vv = stt("vv", s18, -1.0, Z["x"])             # v = -p_x

        def second(name, Ea, Zb, Zdd):
            a1 = stt(name + "_a", Ea, -2.0, Zb)
            a2 = tt(name + "_b", s18, Zdd)
            return tt(name, a1, a2, OP.add)

        p_xx = second("p_xx", e8x, Z["x"], Z["xx"])
        p_xy = second("p_xy", e8x, Z["y"], Z["xy"])
        p_yy = second("p_yy", e8y, Z["y"], Z["yy"])
        p_yt = second("p_yt", e8y, Z["t"], Z["yt"])
        mp_xt_a = stt("mp_xt_a", e8x, 2.0, Z["t"])
        mp_xt_b = stt("mp_xt_b", s18, -1.0, Z["xt"])
        mp_xt = tt("mp_xt", mp_xt_a, mp_xt_b, OP.add)  # -p_xt

        def third3(name, Fa, Za, Ea, Zaa, Zddd):
            a1 = stt(name + "_a", Fa, 2.0, Za)
            a2 = stt(name + "_b", Ea, -6.0, Zaa)
            a3 = tt(name + "_c", s18, Zddd)
            a12 = tt(name + "_ab", a1, a2, OP.add)
            return tt(name, a12, a3, OP.add)

        p_xxx = third3("p_xxx", f8x, Z["x"], e8x, Z["xx"], Z["xxx"])
        p_yyy = third3("p_yyy", f8y, Z["y"], e8y, Z["yy"], Z["yyy"])

        def third_m(name, Fa, Zb, Eb, Zaa, Ea, Zab, Zddd):
            # 2*Fa*Zb - 2*Eb*Zaa - 4*Ea*Zab + s1*Zddd
            a1 = stt(name + "_a", Fa, 2.0, Zb)
            a2 = stt(name + "_b", Eb, -2.0, Zaa)
            a3 = stt(name + "_c", Ea, -4.0, Zab)
            a4 = tt(name + "_d", s18, Zddd)
            a12 = tt(name + "_ab", a1, a2, OP.add)
            a34 = tt(name + "_cd", a3, a4, OP.add)
            return tt(name, a12, a34, OP.add)

        p_xxy = third_m("p_xxy", f8x, Z["y"], e8y, Z["xx"], e8x, Z["xy"],
                        Z["xxy"])
        p_xyy = third_m("p_xyy", f8y, Z["x"], e8x, Z["yy"], e8y, Z["xy"],
                        Z["xyy"])

        # f_u = p_yt + lam1*(u*p_xy + v*p_yy) - lam2*(p_xxy + p_yyy)
        fu_a = tt("fu_a", u, p_xy)
        fu_b = tt("fu_b", vv, p_yy)
        fu_ab = tt("fu_ab", fu_a, fu_b, OP.add)
        fu_l = stt("fu_l", fu_ab, lams[:, 0:1], p_yt, OP.mult, OP.add)
        fu_c = tt("fu_c", p_xxy, p_yyy, OP.add)
        f_u = stt("f_u", fu_c, lams[:, 1:2], fu_l, OP.mult, OP.add)
        # f_v = -p_xt - lam1*(u*p_xx + v*p_xy) + lam2*(p_xxx + p_xyy)
        fv_a = tt("fv_a", u, p_xx)
        fv_b = tt("fv_b", vv, p_xy)
        fv_ab = tt("fv_ab", fv_a, fv_b, OP.add)
        fv_l = stt("fv_l", fv_ab, lams[:, 2:3], mp_xt, OP.mult, OP.add)
        fv_c = tt("fv_c", p_xxx, p_xyy, OP.add)
        f_v = stt("f_v", fv_c, lams[:, 3:4], fv_l, OP.mult, OP.add)

        nc.sync.dma_start(out_d["uo"][:], u[:])
        nc.sync.dma_start(out_d["vo"][:], vv[:])
        nc.sync.dma_start(out_d["fuo"][:], f_u[:])
        nc.sync.dma_start(out_d["fvo"][:], f_v[:])

    return nc


_CACHE = {}


def _get_nc():
    if "nc" not in _CACHE:
        nc = _build()
        nc.finalize()
        _CACHE["nc"] = nc
    return _CACHE["nc"]


def kernel(**inputs):
    nc = _get_nc()
    f32 = np.float32
    x = np.asarray(inputs["x"], f32)[:, 0]
    y = np.asarray(inputs["y"], f32)[:, 0]
    t = np.asarray(inputs["t"], f32)[:, 0]
    pts = np.ascontiguousarray(np.stack([x, y, t], 0))          # [3, N]
    W0 = np.asarray(inputs["W0"], f32)
    cx, cy, ct = W0[0], W0[1], W0[2]
    c0 = np.ascontiguousarray(np.stack(
        [cx, cy, ct,
         cx * cx, cx * cy, cy * cy, cx * ct, cy * ct,
         cx ** 3, cx * cx * cy, cx * cy * cy, cy ** 3],
        1).astype(f32))                                         # [128, 12]
    w8 = np.asarray(inputs["W8"], f32)[:, 0].astype(np.float16)
    W8C = np.zeros([H, 16 * 13], np.float16)
    for s in range(13):
        W8C[:, 16 * s + s] = w8
    lam1 = f32(np.asarray(inputs["lam1"]).reshape(-1)[0])
    lam2 = f32(np.asarray(inputs["lam2"]).reshape(-1)[0])
    shared = {
        "W0f": np.ascontiguousarray(W0),
        "c0": c0,
        "W8C": W8C,
        "b8v": np.full([H, 1], np.asarray(inputs["b8"]).reshape(-1)[0], f32),
        "lam": np.tile(np.array([[lam1, -lam2, -lam1, lam2]], f32), (H, 1)),
    }
    for li in range(1, 8):
        shared[f"Wh{li}"] = np.asarray(inputs[f"W{li}"], f32).astype(
            np.float16)
    for li in range(0, 8):
        shared[f"bb{li}"] = np.asarray(
            inputs[f"b{li}"], f32).reshape(H, 1).copy()

    in_maps = []
    for c in range(N_CORES):
        m = dict(shared)
        m["pts"] = np.ascontiguousarray(pts[:, c * NLOC:(c + 1) * NLOC])
        in_maps.append(m)

    trace = bool(os.environ.get("BASS_KERNEL_TRACE"))
    tdir = os.environ.get("BASS_KERNEL_TRACE_DIR") or None
    res = run_bass_kernel_spmd(nc, in_maps, list(range(N_CORES)),
                               trace=trace, tmpdir=tdir)
    kernel.last_exec_time_ns = res.exec_time_ns
    outs = []
    for name in ["uo", "vo", "fuo", "fvo"]:
        full = np.concatenate(
            [np.asarray(res.results[c][name], f32).reshape(-1)
             for c in range(N_CORES)])
        outs.append(full[:, None])
    return tuple(outs)


kernel.last_exec_time_ns = None
